# revision 6
# baseline (speedup 1.0000x reference)
"""Bundle-adjustment loss kernel for 8 Trainium2 NeuronCores.

Data-parallel over the image axis M: each core processes MC=12800 images
(zero-padded from 100000/8=12500; the len-loss contribution of padded
images is corrected analytically on the host).

Device layout: partition dim = (camera, point) = 96, free dim = images.
Camera constants ride as per-partition [96,1] scalar APs directly in
scalar_tensor_tensor / activation scale+bias slots, so the distortion
polynomial and pixel residuals need no materialized constant planes.
PE does R@X+t as fp16 matmuls W[10,96].T @ XT[10,512] -> PSUM fp32 in
512-image blocks (PSUM bank limit), phase-grouped per coordinate so
redundant ldweights collapse; everything downstream runs on wide
[96, ~6.5K] tiles, almost entirely on the Vector engine.
"""

import numpy as np

M_TOTAL = 100000
C = 32
NCORES = 8
MC = 12800           # images per core (25 blocks of 512)
BLK = 512            # matmul block (one PSUM bank of f32)
SLICES = (6656, 6144)  # wide-op slice widths (13 + 12 blocks)
CP = 96              # (camera, point) pairs
W_LOSS = 0.01        # LINE_W = LEN_W = REPROJ_W
ESCALE = 64.0        # e = (du^2+dv^2)/ESCALE; host multiplies by sqrt(ESCALE)

_NC_CACHE = {}


def _apply_tile_patch():
    """This walrus build rejects Tile's kernel-tail drain carrying every
    semaphore wait on one instruction ("Too many sync wait commands").
    Emit one wait_ge per live semaphore instead."""
    from concourse import tile

    if getattr(tile.TileContext, "_ba_drain_patched", False):
        return

    def _drain_and_barrier(self, tick_clock, wait_clock):
        nc = self.nc
        ticks = list(tick_clock.global_clock)
        allocated = wait_clock.sems.allocated()
        for key, sem in allocated.items():
            t = ticks[int(key)]
            if t > 0:
                nc.sync.wait_ge(sem, t)
        nc.sync.drain()
        nc.all_engine_barrier()
        assert self.sems is not None
        popped = nc._tile_sem_poison_stack.pop()
        assert popped is self._sem_poison
        nc.clear_and_free_semaphores(list(self.sems.allocated().values()))
        nc.all_engine_barrier()

    tile.TileContext._drain_and_barrier = _drain_and_barrier
    tile.TileContext._ba_drain_patched = True


def _spill_excess_waits(nc, cap=1):
    """This walrus build's ISA structs accept very few sync-wait slots
    per compute instruction. Spill waits beyond `cap` onto InstNoOp
    carriers inserted just before the instruction on the same engine."""
    import concourse.mybir as mybir
    import bass_rust

    fragile = {
        "InstTensorScalarPtr", "InstActivation", "InstReciprocal",
        "InstTensorReduce", "InstMatmult", "InstTensorCopy",
        "InstTensorTensor", "InstLdweights", "InstMemset", "InstIota",
        "InstTensorTensorReduce", "InstPool", "InstDMACopy", "InstDMA",
        "InstDmaTransposeAnt",
    }
    n_nop = 0
    for bb in nc.m.functions[0].blocks:
        il = bb.instructions
        out_list = []
        for inst in il:
            si = inst.sync_info
            if (si is not None and type(inst).__name__ in fragile
                    and len(si.on_wait) > cap):
                waits = list(si.on_wait)
                keep, spill = waits[:cap], waits[cap:]
                for wv in spill:
                    nop = mybir.InstNoOp(name=f"ba_waitnop_{n_nop}")
                    n_nop += 1
                    nop.engine = inst.engine
                    nop.sync_info = bass_rust.SyncInfo(
                        on_wait=[wv], on_update=[])
                    out_list.append(nop)
                inst.sync_info = bass_rust.SyncInfo(
                    on_wait=keep, on_update=list(si.on_update))
            out_list.append(inst)
        if len(out_list) != len(il):
            bb.instructions = out_list
    return n_nop


def _ap_key(arg):
    try:
        return str(arg)
    except Exception:
        return repr(arg)


def _dedup_ldweights(nc):
    """Drop InstLdweights whose payload equals the previous ldweights in
    the same block (PE array state is unchanged by intervening matmuls).
    Non-empty sync moves onto an InstNoOp carrier on the same engine."""
    import concourse.mybir as mybir

    n_drop = 0
    for bb in nc.m.functions[0].blocks:
        last = None
        keep = []
        for inst in bb.instructions:
            if type(inst).__name__ == "InstLdweights":
                key = tuple(_ap_key(a) for a in inst.ins)
                if last is not None and key == last:
                    si = inst.sync_info
                    if si is not None and (si.on_wait or si.on_update):
                        nop = mybir.InstNoOp(name=f"ldw_drop_{n_drop}")
                        nop.engine = inst.engine
                        nop.sync_info = si
                        keep.append(nop)
                    n_drop += 1
                    continue
                last = key
            keep.append(inst)
        if n_drop:
            bb.instructions = keep
    return n_drop


def _build_nc(a_coef, b_coef, s_len, reps=1, variant="full"):
    """Build the SPMD Bass module (same program on all 8 cores)."""
    key = (a_coef, b_coef, s_len, reps, variant)
    if key in _NC_CACHE:
        return _NC_CACHE[key]
    import concourse.bass as bass
    import concourse.mybir as mybir
    from concourse import tile

    _apply_tile_patch()
    F32 = mybir.dt.float32
    F16 = mybir.dt.float16
    F8 = mybir.dt.float8e4
    ALU = mybir.AluOpType
    ACT = mybir.ActivationFunctionType

    nc = bass.Bass(trn_type="TRN2")
    # inputs
    obs_u = nc.declare_dram_parameter("obs_u", [CP, MC], F16, isOutput=False)
    obs_v = nc.declare_dram_parameter("obs_v", [CP, MC], F16, isOutput=False)
    maskf = nc.declare_dram_parameter("maskf", [CP, MC], F8, isOutput=False)
    xt = nc.declare_dram_parameter("xt", [10, MC], F16, isOutput=False)
    xn = nc.declare_dram_parameter("xn", [128, 900], F16, isOutput=False)
    wmat = nc.declare_dram_parameter("wmat", [10, 3 * CP], F16, isOutput=False)
    # per-(c,p)-partition camera scalars, one column each:
    # 0:k1 1:k2 2:k3 3:2p1 4:2p2 5:fx*p2 6:fy*p1 7:fx 8:fy 9:-s(len bias)
    cscal = nc.declare_dram_parameter("cscal", [128, 16], F32, isOutput=False)
    out = nc.declare_dram_parameter("out", [224], F32, isOutput=True)

    WMAX = max(SLICES)
    NSL = len(SLICES)

    with tile.TileContext(nc) as tc:
        with (
            tc.tile_pool(name="sb", bufs=1) as sb,
            tc.tile_pool(name="psum", bufs=1,
                         space=bass.MemorySpace.PSUM) as pp,
        ):
            # --- resident tiles (allocated once) ---
            xt_t = sb.tile([10, MC], F16, tag="xt", name="xt_t")
            xn_t = sb.tile([128, 900], F16, tag="xn", name="xn_t")
            wm_t = sb.tile([10, 3 * CP], F16, tag="wm", name="wm_t")
            cs_t = sb.tile([128, 16], F32, tag="cs", name="cs_t")
            ou_t = sb.tile([CP, WMAX], F16, tag="ou", name="ou_t")
            ov_t = sb.tile([CP, WMAX], F16, tag="ov", name="ov_t")
            mk_t = sb.tile([CP, WMAX], F8, tag="mk", name="mk_t")
            x0p = sb.tile([CP, WMAX], F16, tag="x0p", name="x0p")
            x1p = sb.tile([CP, WMAX], F16, tag="x1p", name="x1p")
            izp = sb.tile([CP, WMAX], F32, tag="izp", name="izp")
            wA = sb.tile([CP, WMAX], F16, tag="wA", name="wA")
            wB = sb.tile([CP, WMAX], F16, tag="wB", name="wB")
            wC = sb.tile([CP, WMAX], F16, tag="wC", name="wC")
            wD = sb.tile([CP, WMAX], F16, tag="wD", name="wD")
            wE = sb.tile([CP, WMAX], F16, tag="wE", name="wE")
            # line/len working tiles
            tb_t = sb.tile([128, 300], F16, tag="tb", name="tb_t")
            g_t = sb.tile([128, 300], F16, tag="g", name="g_t")
            dc_t = sb.tile([128, 600], F16, tag="dc", name="dc_t")
            sq_t = sb.tile([128, 600], F16, tag="sq", name="sq_t")
            rd_t = sb.tile([128, 200], F32, tag="rd", name="rd_t")
            rt_t = sb.tile([128, 200], F16, tag="rt", name="rt_t")
            ln_t = sb.tile([128, 100], F16, tag="ln", name="ln_t")
            cb_t = sb.tile([128, 100], F16, tag="cb", name="cb_t")
            # stages
            pt_stage = sb.tile([CP, NSL], F32, tag="pts", name="pt_stage")
            ll_stage = sb.tile([128, 1], F32, tag="lls", name="ll_stage")
            ptred = sb.tile([CP, 1], F32, tag="ptr", name="ptred")
            # psum: one tile per bank, manual ring
            ps = [pp.tile([CP, BLK], F32, tag=f"ps{k}", name=f"ps{k}")
                  for k in range(8)]

            # camera-scalar column APs
            k1s = cs_t[0:CP, 0:1]
            k2s = cs_t[0:CP, 1:2]
            k3s = cs_t[0:CP, 2:3]
            tp1s = cs_t[0:CP, 3:4]
            tp2s = cs_t[0:CP, 4:5]
            fxp2s = cs_t[0:CP, 5:6]
            fyp1s = cs_t[0:CP, 6:7]
            fxs = cs_t[0:CP, 7:8]
            fys = cs_t[0:CP, 8:9]
            negs = cs_t[:, 9:10]

            # prologue loads
            nc.sync.dma_start(wm_t[:], wmat[:])
            nc.sync.dma_start(cs_t[:], cscal[:])

            for _rep in range(reps):
                nc.sync.dma_start(xt_t[:], xt[:])
                nc.sync.dma_start(xn_t[:], xn[:])

                if variant == "dmaonly":
                    nc.sync.dma_start(ou_t[:], obs_u[:, 0:WMAX])
                    nc.vector.tensor_reduce(
                        pt_stage[:, 0:1], ou_t[:],
                        mybir.AxisListType.X, ALU.add)
                    nc.vector.tensor_reduce(
                        ll_stage[:], xn_t[:].rearrange(
                            "p (a j) -> p a j", a=100)[:, :, 0:3],
                        mybir.AxisListType.XY, ALU.add)
                    continue

                # ---- line/len losses (image-partition pipeline) ----
                xnv = xn_t[:].rearrange("p (a j) -> p a j", a=100)
                x0s, x1s, x2s = xnv[:, :, 0:3], xnv[:, :, 3:6], xnv[:, :, 6:9]
                tbv = tb_t[:].rearrange("p (a j) -> p a j", a=100)
                gv = g_t[:].rearrange("p (a j) -> p a j", a=100)
                dcv = dc_t[:].rearrange("p (a k j) -> p a k j", a=100, k=2)
                nc.vector.tensor_scalar_mul(tbv, x2s, b_coef)
                nc.vector.scalar_tensor_tensor(
                    gv, x0s, a_coef, tbv, ALU.mult, ALU.add)
                nc.vector.scalar_tensor_tensor(
                    dcv[:, :, 0, :], gv, -1.0, x1s, ALU.mult, ALU.add)
                nc.vector.scalar_tensor_tensor(
                    dcv[:, :, 1, :], x0s, 1.0, x2s, ALU.mult, ALU.subtract)
                nc.vector.scalar_tensor_tensor(
                    sq_t[:], dc_t[:], 1.0, dc_t[:], ALU.mult, ALU.mult)
                nc.vector.tensor_reduce(
                    rd_t[:].rearrange("p (a k) -> p a k", k=2),
                    sq_t[:].rearrange("p (a k j) -> p a k j", a=100, k=2),
                    mybir.AxisListType.X, ALU.add)
                nc.scalar.activation(rt_t[:], rd_t[:], ACT.Sqrt)
                rtv = rt_t[:].rearrange("p (a k) -> p a k", k=2)
                nc.scalar.activation(ln_t[:], rtv[:, :, 1], ACT.Abs,
                                     bias=negs)
                nc.vector.scalar_tensor_tensor(
                    cb_t[:], rtv[:, :, 0], 1.0, ln_t[:], ALU.mult, ALU.add)
                nc.vector.tensor_reduce(
                    ll_stage[:], cb_t[:], mybir.AxisListType.X, ALU.add)

                # ---- reprojection loss ----
                base = 0
                for sl, W in enumerate(SLICES):
                    nblk = W // BLK
                    nc.sync.dma_start(ou_t[:, 0:W], obs_u[:, base:base + W])
                    nc.sync.dma_start(ov_t[:, 0:W], obs_v[:, base:base + W])
                    nc.sync.dma_start(mk_t[:, 0:W], maskf[:, base:base + W])

                    gens = [list(range(g, min(g + 8, nblk)))
                            for g in range(0, nblk, 8)]

                    def emit_phase(wsl, consume):
                        # mm-then-consume in generations of <=8 blocks so a
                        # bank is read before a later block overwrites it
                        for gen in gens:
                            for b in gen:
                                g0 = base + b * BLK
                                nc.tensor.matmul(ps[b % 8][:], wsl,
                                                 xt_t[:, g0:g0 + BLK])
                            for b in gen:
                                consume(b * BLK, ps[b % 8])

                    # Z phase -> izp, then X -> x0p, then Y -> x1p
                    def c_iz(o, bank):
                        nc.vector.reciprocal(izp[:, o:o + BLK], bank[:])

                    def c_x0(o, bank):
                        nc.vector.scalar_tensor_tensor(
                            x0p[:, o:o + BLK], bank[:], 1.0,
                            izp[:, o:o + BLK], ALU.mult, ALU.mult)

                    def c_x1(o, bank):
                        nc.vector.scalar_tensor_tensor(
                            x1p[:, o:o + BLK], bank[:], 1.0,
                            izp[:, o:o + BLK], ALU.mult, ALU.mult)

                    emit_phase(wm_t[:, 2 * CP:3 * CP], c_iz)
                    emit_phase(wm_t[:, 0:CP], c_x0)
                    emit_phase(wm_t[:, CP:2 * CP], c_x1)

                    # wide ops on [CP, W] (Vector engine except P1 + sqrt)
                    x0 = x0p[:, 0:W]
                    x1 = x1p[:, 0:W]
                    A, B, Cw, D, E = (t[:, 0:W] for t in (wA, wB, wC, wD, wE))
                    ou = ou_t[:, 0:W]
                    ov = ov_t[:, 0:W]
                    mk = mk_t[:, 0:W]
                    nc.vector.scalar_tensor_tensor(            # x0n^2
                        A, x0, 1.0, x0, ALU.mult, ALU.mult)
                    nc.vector.scalar_tensor_tensor(            # x1n^2
                        B, x1, 1.0, x1, ALU.mult, ALU.mult)
                    nc.vector.scalar_tensor_tensor(            # r2
                        Cw, A, 1.0, B, ALU.mult, ALU.add)
                    nc.scalar.activation(A, Cw, ACT.Identity,  # k3*r2+k2
                                         bias=k2s, scale=k3s)
                    nc.vector.scalar_tensor_tensor(            # *r2
                        B, A, 1.0, Cw, ALU.mult, ALU.mult)
                    nc.vector.scalar_tensor_tensor(            # (+k1)*r2
                        A, B, k1s, Cw, ALU.add, ALU.mult)
                    nc.vector.scalar_tensor_tensor(            # +2p1*x1n
                        B, x1, tp1s, A, ALU.mult, ALU.add)
                    nc.vector.scalar_tensor_tensor(            # ra-1
                        A, x0, tp2s, B, ALU.mult, ALU.add)
                    nc.vector.scalar_tensor_tensor(            # mu
                        B, A, 1.0, x0, ALU.add, ALU.mult)
                    nc.vector.scalar_tensor_tensor(            # mv
                        D, A, 1.0, x1, ALU.add, ALU.mult)
                    nc.vector.scalar_tensor_tensor(            # fx*mu-ou
                        E, B, fxs, ou, ALU.mult, ALU.subtract)
                    nc.vector.scalar_tensor_tensor(            # du=^+fxp2*r2
                        B, Cw, fxp2s, E, ALU.mult, ALU.add)
                    nc.vector.scalar_tensor_tensor(            # fy*mv-ov
                        E, D, fys, ov, ALU.mult, ALU.subtract)
                    nc.vector.scalar_tensor_tensor(            # dv=^+fyp1*r2
                        D, Cw, fyp1s, E, ALU.mult, ALU.add)
                    nc.vector.scalar_tensor_tensor(            # du^2/ESCALE
                        A, B, 1.0 / ESCALE, B, ALU.mult, ALU.mult)
                    nc.vector.scalar_tensor_tensor(            # dv^2/ESCALE
                        E, D, 1.0 / ESCALE, D, ALU.mult, ALU.mult)
                    nc.vector.scalar_tensor_tensor(            # e
                        B, A, 1.0, E, ALU.mult, ALU.add)
                    nc.vector.scalar_tensor_tensor(            # e*mask
                        Cw, B, 1.0, mk, ALU.mult, ALU.mult)
                    nc.scalar.activation(A, Cw, ACT.Sqrt,      # sum
                                         accum_out=pt_stage[:, sl:sl + 1])
                    base += W

            # ---- epilogue ----
            nc.vector.tensor_reduce(ptred[:], pt_stage[:],
                                    mybir.AxisListType.X, ALU.add)
            nc.sync.dma_start(out[0:CP], ptred[:])
            nc.sync.dma_start(out[CP:224], ll_stage[:])

    _dedup_ldweights(nc)
    _spill_excess_waits(nc)
    _NC_CACHE[key] = nc
    return nc


def kernel(pole, pole_3ds, pole_2ds, mask, K, dist, R, t):
    import ml_dtypes

    pole = np.asarray(pole, np.float32)
    pole_3ds = np.asarray(pole_3ds, np.float32)
    pole_2ds = np.asarray(pole_2ds, np.float32)
    mask = np.asarray(mask)
    K = np.asarray(K, np.float32)
    dist = np.asarray(dist, np.float32)
    R = np.asarray(R, np.float32)
    t = np.asarray(t, np.float32)

    s = float(pole[0] + pole[1])
    a_coef = float(pole[1] / s)   # coefficient of X0 in exp_p1
    b_coef = float(pole[0] / s)   # coefficient of X2

    def rep(v):  # [C] -> [CP]
        return np.repeat(v.astype(np.float32), 3)

    # per-partition camera scalars
    cscal = np.zeros((128, 16), np.float32)
    cscal[:CP, 0] = rep(dist[:, 0])                 # k1
    cscal[:CP, 1] = rep(dist[:, 1])                 # k2
    cscal[:CP, 2] = rep(dist[:, 4])                 # k3
    cscal[:CP, 3] = rep(2.0 * dist[:, 2])           # 2*p1
    cscal[:CP, 4] = rep(2.0 * dist[:, 3])           # 2*p2
    cscal[:CP, 5] = rep(K[:, 0, 0] * dist[:, 3])    # fx*p2
    cscal[:CP, 6] = rep(K[:, 1, 1] * dist[:, 2])    # fy*p1
    cscal[:CP, 7] = rep(K[:, 0, 0])                 # fx
    cscal[:CP, 8] = rep(K[:, 1, 1])                 # fy
    cscal[:, 9] = -s                                # len-loss bias
    u0_cp = rep(K[:, 0, 2])   # [CP]
    v0_cp = rep(K[:, 1, 2])

    # matmul weights: wmat[j, c*96 + (cam*3+p)] for coordinate c
    wbase = np.zeros((3, 10, CP), np.float32)
    for p in range(3):
        wbase[:, p * 3:p * 3 + 3, p::3] = R.transpose(1, 2, 0)
    wbase[:, 9, :] = np.repeat(t.T, 3, axis=1)
    wmat = np.ascontiguousarray(
        wbase.transpose(1, 0, 2).reshape(10, 3 * CP)).astype(np.float16)

    # shard + pad the big tensors
    mc_all = NCORES * MC
    npad = mc_all - M_TOTAL
    in_maps = []
    for core in range(NCORES):
        ms, me = core * 12500, (core + 1) * 12500
        n = me - ms
        p3 = pole_3ds[ms:me].reshape(n, 9)
        xt = np.zeros((10, MC), np.float16)
        xt[:9, :n] = p3.T.astype(np.float16)
        xt[9, :] = 1.0
        ou = np.zeros((CP, MC), np.float16)
        ov = np.zeros((CP, MC), np.float16)
        ou[:, :n] = (pole_2ds[ms:me, :, :, 0].reshape(n, CP)
                     - u0_cp[None, :]).T
        ov[:, :n] = (pole_2ds[ms:me, :, :, 1].reshape(n, CP)
                     - v0_cp[None, :]).T
        mk = np.zeros((CP, MC), np.float16)
        mk[:, :n] = np.repeat(mask[ms:me].astype(np.float16), 3, axis=1).T
        # line/len layout: image m_local = p*100 + a -> xn[p, 9a:9a+9]
        xnat = np.zeros((MC, 9), np.float16)
        xnat[:n] = p3.astype(np.float16)
        xn = np.ascontiguousarray(xnat.reshape(128, 900))
        in_maps.append({
            "obs_u": ou, "obs_v": ov,
            "maskf": mk.astype(ml_dtypes.float8_e4m3),
            "xt": xt, "xn": xn, "wmat": wmat, "cscal": cscal,
        })

    nc = _build_nc(a_coef, b_coef, s)

    from concourse.bass_utils import run_bass_kernel_spmd
    res = run_bass_kernel_spmd(nc, in_maps, core_ids=list(range(NCORES)))
    pt_sum = 0.0
    ll_sum = 0.0
    for r in res.results:
        o = np.asarray(r["out"], np.float64)
        pt_sum += o[0:CP].sum()
        ll_sum += o[CP:224].sum()
    # padded images contribute |0 - s| = s to the len loss each
    loss = W_LOSS * (np.sqrt(ESCALE) * pt_sum + ll_sum - npad * s) / M_TOTAL
    return np.float32(loss)


# revision 8
# speedup vs baseline: 3.2032x; 3.2032x over previous
"""Bundle-adjustment loss kernel for 8 Trainium2 NeuronCores.

Data-parallel over the image axis M: each core processes MC=12800 images
(zero-padded from 100000/8=12500; the len-loss contribution of padded
images is corrected analytically on the host).

Device layout: partition dim = (camera, point) = 96, free dim = images.
Camera constants ride as per-partition [96,1] scalar APs directly in
scalar_tensor_tensor / activation scale+bias slots, so the distortion
polynomial and pixel residuals need no materialized constant planes.
PE does R@X+t as fp16 matmuls W[10,96].T @ XT[10,512] -> PSUM fp32 in
512-image blocks (PSUM bank limit), phase-grouped per coordinate so
redundant ldweights collapse; everything downstream runs on wide
[96, ~6.5K] tiles, almost entirely on the Vector engine.
"""

import numpy as np

M_TOTAL = 100000
C = 32
NCORES = 8
MC = 12800           # images per core (25 blocks of 512)
BLK = 512            # matmul block (one PSUM bank of f32)
SLICES = (6656, 6144)  # wide-op slice widths (13 + 12 blocks)
CP = 96              # (camera, point) pairs
W_LOSS = 0.01        # LINE_W = LEN_W = REPROJ_W
ESCALE = 64.0        # e = (du^2+dv^2)/ESCALE; host multiplies by sqrt(ESCALE)

_NC_CACHE = {}


def _apply_tile_patch():
    """This walrus build rejects Tile's kernel-tail drain carrying every
    semaphore wait on one instruction ("Too many sync wait commands").
    Emit one wait_ge per live semaphore instead."""
    from concourse import tile

    if getattr(tile.TileContext, "_ba_drain_patched", False):
        return

    def _drain_and_barrier(self, tick_clock, wait_clock):
        nc = self.nc
        ticks = list(tick_clock.global_clock)
        allocated = wait_clock.sems.allocated()
        for key, sem in allocated.items():
            t = ticks[int(key)]
            if t > 0:
                nc.sync.wait_ge(sem, t)
        nc.sync.drain()
        nc.all_engine_barrier()
        assert self.sems is not None
        popped = nc._tile_sem_poison_stack.pop()
        assert popped is self._sem_poison
        nc.clear_and_free_semaphores(list(self.sems.allocated().values()))
        nc.all_engine_barrier()

    tile.TileContext._drain_and_barrier = _drain_and_barrier
    tile.TileContext._ba_drain_patched = True


def _spill_excess_waits(nc, cap=1):
    """This walrus build's ISA structs accept very few sync-wait slots
    per compute instruction. Spill waits beyond `cap` onto InstNoOp
    carriers inserted just before the instruction on the same engine."""
    import concourse.mybir as mybir
    import bass_rust

    fragile = {
        "InstTensorScalarPtr", "InstActivation", "InstReciprocal",
        "InstTensorReduce", "InstMatmult", "InstTensorCopy",
        "InstTensorTensor", "InstLdweights", "InstMemset", "InstIota",
        "InstTensorTensorReduce", "InstPool", "InstDMACopy", "InstDMA",
        "InstDmaTransposeAnt",
    }
    n_nop = 0
    for bb in nc.m.functions[0].blocks:
        il = bb.instructions
        out_list = []
        for inst in il:
            si = inst.sync_info
            if (si is not None and type(inst).__name__ in fragile
                    and len(si.on_wait) > cap):
                waits = list(si.on_wait)
                keep, spill = waits[:cap], waits[cap:]
                for wv in spill:
                    nop = mybir.InstNoOp(name=f"ba_waitnop_{n_nop}")
                    n_nop += 1
                    nop.engine = inst.engine
                    nop.sync_info = bass_rust.SyncInfo(
                        on_wait=[wv], on_update=[])
                    out_list.append(nop)
                inst.sync_info = bass_rust.SyncInfo(
                    on_wait=keep, on_update=list(si.on_update))
            out_list.append(inst)
        if len(out_list) != len(il):
            bb.instructions = out_list
    return n_nop


def _ap_key(arg):
    try:
        return str(arg)
    except Exception:
        return repr(arg)


def _dedup_ldweights(nc):
    """Drop InstLdweights whose payload equals the previous ldweights in
    the same block (PE array state is unchanged by intervening matmuls).
    Non-empty sync moves onto an InstNoOp carrier on the same engine."""
    import concourse.mybir as mybir

    n_drop = 0
    for bb in nc.m.functions[0].blocks:
        last = None
        keep = []
        for inst in bb.instructions:
            if type(inst).__name__ == "InstLdweights":
                key = tuple(_ap_key(a) for a in inst.ins)
                if last is not None and key == last:
                    si = inst.sync_info
                    if si is not None and (si.on_wait or si.on_update):
                        nop = mybir.InstNoOp(name=f"ldw_drop_{n_drop}")
                        nop.engine = inst.engine
                        nop.sync_info = si
                        keep.append(nop)
                    n_drop += 1
                    continue
                last = key
            keep.append(inst)
        if n_drop:
            bb.instructions = keep
    return n_drop


def _build_nc(a_coef, b_coef, s_len, reps=1, variant="full"):
    """Build the SPMD Bass module (same program on all 8 cores)."""
    key = (a_coef, b_coef, s_len, reps, variant)
    if key in _NC_CACHE:
        return _NC_CACHE[key]
    import concourse.bass as bass
    import concourse.mybir as mybir
    from concourse import tile

    _apply_tile_patch()
    F32 = mybir.dt.float32
    F16 = mybir.dt.float16
    F8 = mybir.dt.float8e4
    ALU = mybir.AluOpType
    ACT = mybir.ActivationFunctionType

    nc = bass.Bass(trn_type="TRN2")
    # inputs
    obs_u = nc.declare_dram_parameter("obs_u", [CP, MC], F16, isOutput=False)
    obs_v = nc.declare_dram_parameter("obs_v", [CP, MC], F16, isOutput=False)
    maskf = nc.declare_dram_parameter("maskf", [CP, MC], F8, isOutput=False)
    xt = nc.declare_dram_parameter("xt", [10, MC], F16, isOutput=False)
    xn = nc.declare_dram_parameter("xn", [128, 900], F16, isOutput=False)
    wmat = nc.declare_dram_parameter("wmat", [10, 3 * CP], F16, isOutput=False)
    # per-(c,p)-partition camera scalars, one column each:
    # 0:k1 1:k2 2:k3 3:2p1 4:2p2 5:fx*p2 6:fy*p1 7:fx 8:fy 9:-s(len bias)
    cscal = nc.declare_dram_parameter("cscal", [128, 16], F32, isOutput=False)
    out = nc.declare_dram_parameter("out", [224], F32, isOutput=True)

    WMAX = max(SLICES)
    NSL = len(SLICES)

    with tile.TileContext(nc) as tc:
        with (
            tc.tile_pool(name="sb", bufs=1) as sb,
            tc.tile_pool(name="psum", bufs=1,
                         space=bass.MemorySpace.PSUM) as pp,
        ):
            # --- resident tiles (allocated once) ---
            xt_t = sb.tile([10, MC], F16, tag="xt", name="xt_t")
            xn_t = sb.tile([128, 900], F16, tag="xn", name="xn_t")
            wm_t = sb.tile([10, 3 * CP], F16, tag="wm", name="wm_t")
            cs_t = sb.tile([128, 16], F32, tag="cs", name="cs_t")
            ou_t = sb.tile([CP, WMAX], F16, tag="ou", name="ou_t")
            ov_t = sb.tile([CP, WMAX], F16, tag="ov", name="ov_t")
            mk_t = sb.tile([CP, WMAX], F8, tag="mk", name="mk_t")
            x0p = sb.tile([CP, WMAX], F16, tag="x0p", name="x0p")
            x1p = sb.tile([CP, WMAX], F16, tag="x1p", name="x1p")
            izp = sb.tile([CP, WMAX], F32, tag="izp", name="izp")
            wA = sb.tile([CP, WMAX], F16, tag="wA", name="wA")
            wB = sb.tile([CP, WMAX], F16, tag="wB", name="wB")
            wC = sb.tile([CP, WMAX], F16, tag="wC", name="wC")
            wD = sb.tile([CP, WMAX], F16, tag="wD", name="wD")
            wE = sb.tile([CP, WMAX], F16, tag="wE", name="wE")
            # line/len working tiles
            g_t = sb.tile([128, 300], F16, tag="g", name="g_t")
            dc_t = sb.tile([128, 600], F16, tag="dc", name="dc_t")
            sq_t = sb.tile([128, 600], F16, tag="sq", name="sq_t")
            rd_t = sb.tile([128, 200], F32, tag="rd", name="rd_t")
            rt_t = sb.tile([128, 200], F16, tag="rt", name="rt_t")
            ln_t = sb.tile([128, 100], F16, tag="ln", name="ln_t")
            cb_t = sb.tile([128, 100], F16, tag="cb", name="cb_t")
            # stages
            pt_stage = sb.tile([CP, NSL], F32, tag="pts", name="pt_stage")
            ll_stage = sb.tile([128, 1], F32, tag="lls", name="ll_stage")
            ptred = sb.tile([CP, 1], F32, tag="ptr", name="ptred")
            # psum: one tile per bank, manual ring
            ps = [pp.tile([CP, BLK], F32, tag=f"ps{k}", name=f"ps{k}")
                  for k in range(8)]

            # camera-scalar column APs
            k1s = cs_t[0:CP, 0:1]
            k2s = cs_t[0:CP, 1:2]
            k3s = cs_t[0:CP, 2:3]
            tp1s = cs_t[0:CP, 3:4]
            tp2s = cs_t[0:CP, 4:5]
            fxp2s = cs_t[0:CP, 5:6]
            fyp1s = cs_t[0:CP, 6:7]
            fxs = cs_t[0:CP, 7:8]
            fys = cs_t[0:CP, 8:9]
            negs = cs_t[:, 9:10]

            # prologue loads
            nc.sync.dma_start(wm_t[:], wmat[:])
            nc.sync.dma_start(cs_t[:], cscal[:])

            for _rep in range(reps):
                nc.sync.dma_start(xt_t[:], xt[:])
                nc.sync.dma_start(xn_t[:], xn[:])

                if variant == "dmaonly":
                    nc.sync.dma_start(ou_t[:], obs_u[:, 0:WMAX])
                    nc.vector.tensor_reduce(
                        pt_stage[:, 0:1], ou_t[:],
                        mybir.AxisListType.X, ALU.add)
                    nc.vector.tensor_reduce(
                        ll_stage[:], xn_t[:].rearrange(
                            "p (a j) -> p a j", a=100)[:, :, 0:3],
                        mybir.AxisListType.XY, ALU.add)
                    continue

                # ---- line/len losses (image-partition pipeline) ----
                xnv = xn_t[:].rearrange("p (a j) -> p a j", a=100)
                x0s, x1s, x2s = xnv[:, :, 0:3], xnv[:, :, 3:6], xnv[:, :, 6:9]
                gv = g_t[:].rearrange("p (a j) -> p a j", a=100)
                dcv = dc_t[:].rearrange("p (a k j) -> p a k j", a=100, k=2)
                # line diff via 2 fused affines: (a*x0 - x1) + b*x2
                nc.vector.scalar_tensor_tensor(
                    gv, x0s, a_coef, x1s, ALU.mult, ALU.subtract)
                nc.vector.scalar_tensor_tensor(
                    dcv[:, :, 0, :], x2s, b_coef, gv, ALU.mult, ALU.add)
                nc.vector.scalar_tensor_tensor(
                    dcv[:, :, 1, :], x0s, 1.0, x2s, ALU.mult, ALU.subtract)
                nc.vector.scalar_tensor_tensor(
                    sq_t[:], dc_t[:], 1.0, dc_t[:], ALU.mult, ALU.mult)
                nc.vector.tensor_reduce(
                    rd_t[:].rearrange("p (a k) -> p a k", k=2),
                    sq_t[:].rearrange("p (a k j) -> p a k j", a=100, k=2),
                    mybir.AxisListType.X, ALU.add)
                nc.scalar.activation(rt_t[:], rd_t[:], ACT.Sqrt)
                rtv = rt_t[:].rearrange("p (a k) -> p a k", k=2)
                nc.scalar.activation(ln_t[:], rtv[:, :, 1], ACT.Abs,
                                     bias=negs)
                nc.vector.scalar_tensor_tensor(
                    cb_t[:], rtv[:, :, 0], 1.0, ln_t[:], ALU.mult, ALU.add)
                nc.vector.tensor_reduce(
                    ll_stage[:], cb_t[:], mybir.AxisListType.X, ALU.add)

                # ---- reprojection loss ----
                base = 0
                for sl, W in enumerate(SLICES):
                    nblk = W // BLK
                    nc.sync.dma_start(ou_t[:, 0:W], obs_u[:, base:base + W])
                    nc.sync.dma_start(ov_t[:, 0:W], obs_v[:, base:base + W])
                    nc.sync.dma_start(mk_t[:, 0:W], maskf[:, base:base + W])

                    gens = [list(range(g, min(g + 8, nblk)))
                            for g in range(0, nblk, 8)]

                    def emit_phase(wsl, consume):
                        # mm-then-consume in generations of <=8 blocks so a
                        # bank is read before a later block overwrites it
                        for gen in gens:
                            for b in gen:
                                g0 = base + b * BLK
                                nc.tensor.matmul(ps[b % 8][:], wsl,
                                                 xt_t[:, g0:g0 + BLK])
                            for b in gen:
                                consume(b * BLK, ps[b % 8])

                    # Z phase -> izp, then X -> x0p, then Y -> x1p
                    def c_iz(o, bank):
                        nc.vector.reciprocal(izp[:, o:o + BLK], bank[:])

                    def c_x0(o, bank):
                        nc.vector.scalar_tensor_tensor(
                            x0p[:, o:o + BLK], bank[:], 1.0,
                            izp[:, o:o + BLK], ALU.mult, ALU.mult)

                    def c_x1(o, bank):
                        nc.vector.scalar_tensor_tensor(
                            x1p[:, o:o + BLK], bank[:], 1.0,
                            izp[:, o:o + BLK], ALU.mult, ALU.mult)

                    emit_phase(wm_t[:, 2 * CP:3 * CP], c_iz)
                    emit_phase(wm_t[:, 0:CP], c_x0)
                    emit_phase(wm_t[:, CP:2 * CP], c_x1)

                    # wide ops on [CP, W] (Vector engine except P1 + sqrt)
                    x0 = x0p[:, 0:W]
                    x1 = x1p[:, 0:W]
                    A, B, Cw, D, E = (t[:, 0:W] for t in (wA, wB, wC, wD, wE))
                    ou = ou_t[:, 0:W]
                    ov = ov_t[:, 0:W]
                    mk = mk_t[:, 0:W]
                    nc.vector.scalar_tensor_tensor(            # x0n^2
                        A, x0, 1.0, x0, ALU.mult, ALU.mult)
                    nc.vector.scalar_tensor_tensor(            # x1n^2
                        B, x1, 1.0, x1, ALU.mult, ALU.mult)
                    nc.vector.scalar_tensor_tensor(            # r2
                        Cw, A, 1.0, B, ALU.mult, ALU.add)
                    nc.scalar.activation(A, Cw, ACT.Identity,  # k3*r2+k2
                                         bias=k2s, scale=k3s)
                    nc.vector.scalar_tensor_tensor(            # *r2
                        B, A, 1.0, Cw, ALU.mult, ALU.mult)
                    nc.vector.scalar_tensor_tensor(            # (+k1)*r2
                        A, B, k1s, Cw, ALU.add, ALU.mult)
                    nc.vector.scalar_tensor_tensor(            # +2p1*x1n
                        B, x1, tp1s, A, ALU.mult, ALU.add)
                    nc.vector.scalar_tensor_tensor(            # ra-1
                        A, x0, tp2s, B, ALU.mult, ALU.add)
                    nc.vector.scalar_tensor_tensor(            # mu
                        B, A, 1.0, x0, ALU.add, ALU.mult)
                    nc.vector.scalar_tensor_tensor(            # mv
                        D, A, 1.0, x1, ALU.add, ALU.mult)
                    nc.vector.scalar_tensor_tensor(            # fx*mu-ou
                        E, B, fxs, ou, ALU.mult, ALU.subtract)
                    nc.vector.scalar_tensor_tensor(            # du=^+fxp2*r2
                        B, Cw, fxp2s, E, ALU.mult, ALU.add)
                    nc.vector.scalar_tensor_tensor(            # fy*mv-ov
                        E, D, fys, ov, ALU.mult, ALU.subtract)
                    nc.vector.scalar_tensor_tensor(            # dv=^+fyp1*r2
                        D, Cw, fyp1s, E, ALU.mult, ALU.add)
                    nc.vector.scalar_tensor_tensor(            # du^2/ESCALE
                        A, B, 1.0 / ESCALE, B, ALU.mult, ALU.mult)
                    nc.vector.scalar_tensor_tensor(            # dv^2/ESCALE
                        E, D, 1.0 / ESCALE, D, ALU.mult, ALU.mult)
                    nc.vector.scalar_tensor_tensor(            # e
                        B, A, 1.0, E, ALU.mult, ALU.add)
                    nc.vector.scalar_tensor_tensor(            # e*mask
                        Cw, B, 1.0, mk, ALU.mult, ALU.mult)
                    nc.scalar.activation(A, Cw, ACT.Sqrt,      # sum
                                         accum_out=pt_stage[:, sl:sl + 1])
                    base += W

            # ---- epilogue ----
            nc.vector.tensor_reduce(ptred[:], pt_stage[:],
                                    mybir.AxisListType.X, ALU.add)
            nc.sync.dma_start(out[0:CP], ptred[:])
            nc.sync.dma_start(out[CP:224], ll_stage[:])

    _dedup_ldweights(nc)
    _spill_excess_waits(nc)
    _NC_CACHE[key] = nc
    return nc


def kernel(pole, pole_3ds, pole_2ds, mask, K, dist, R, t):
    import ml_dtypes

    pole = np.asarray(pole, np.float32)
    pole_3ds = np.asarray(pole_3ds, np.float32)
    pole_2ds = np.asarray(pole_2ds, np.float32)
    mask = np.asarray(mask)
    K = np.asarray(K, np.float32)
    dist = np.asarray(dist, np.float32)
    R = np.asarray(R, np.float32)
    t = np.asarray(t, np.float32)

    s = float(pole[0] + pole[1])
    a_coef = float(pole[1] / s)   # coefficient of X0 in exp_p1
    b_coef = float(pole[0] / s)   # coefficient of X2

    def rep(v):  # [C] -> [CP]
        return np.repeat(v.astype(np.float32), 3)

    # per-partition camera scalars
    cscal = np.zeros((128, 16), np.float32)
    cscal[:CP, 0] = rep(dist[:, 0])                 # k1
    cscal[:CP, 1] = rep(dist[:, 1])                 # k2
    cscal[:CP, 2] = rep(dist[:, 4])                 # k3
    cscal[:CP, 3] = rep(2.0 * dist[:, 2])           # 2*p1
    cscal[:CP, 4] = rep(2.0 * dist[:, 3])           # 2*p2
    cscal[:CP, 5] = rep(K[:, 0, 0] * dist[:, 3])    # fx*p2
    cscal[:CP, 6] = rep(K[:, 1, 1] * dist[:, 2])    # fy*p1
    cscal[:CP, 7] = rep(K[:, 0, 0])                 # fx
    cscal[:CP, 8] = rep(K[:, 1, 1])                 # fy
    cscal[:, 9] = -s                                # len-loss bias
    u0_cp = rep(K[:, 0, 2])   # [CP]
    v0_cp = rep(K[:, 1, 2])

    # matmul weights: wmat[j, c*96 + (cam*3+p)] for coordinate c
    wbase = np.zeros((3, 10, CP), np.float32)
    for p in range(3):
        wbase[:, p * 3:p * 3 + 3, p::3] = R.transpose(1, 2, 0)
    wbase[:, 9, :] = np.repeat(t.T, 3, axis=1)
    wmat = np.ascontiguousarray(
        wbase.transpose(1, 0, 2).reshape(10, 3 * CP)).astype(np.float16)

    # shard + pad the big tensors
    mc_all = NCORES * MC
    npad = mc_all - M_TOTAL
    in_maps = []
    for core in range(NCORES):
        ms, me = core * 12500, (core + 1) * 12500
        n = me - ms
        p3 = pole_3ds[ms:me].reshape(n, 9)
        xt = np.zeros((10, MC), np.float16)
        xt[:9, :n] = p3.T.astype(np.float16)
        xt[9, :] = 1.0
        ou = np.zeros((CP, MC), np.float16)
        ov = np.zeros((CP, MC), np.float16)
        ou[:, :n] = (pole_2ds[ms:me, :, :, 0].reshape(n, CP)
                     - u0_cp[None, :]).T
        ov[:, :n] = (pole_2ds[ms:me, :, :, 1].reshape(n, CP)
                     - v0_cp[None, :]).T
        mk = np.zeros((CP, MC), np.float16)
        mk[:, :n] = np.repeat(mask[ms:me].astype(np.float16), 3, axis=1).T
        # line/len layout: image m_local = p*100 + a -> xn[p, 9a:9a+9]
        xnat = np.zeros((MC, 9), np.float16)
        xnat[:n] = p3.astype(np.float16)
        xn = np.ascontiguousarray(xnat.reshape(128, 900))
        in_maps.append({
            "obs_u": ou, "obs_v": ov,
            "maskf": mk.astype(ml_dtypes.float8_e4m3),
            "xt": xt, "xn": xn, "wmat": wmat, "cscal": cscal,
        })

    nc = _build_nc(a_coef, b_coef, s)

    from concourse.bass_utils import run_bass_kernel_spmd
    res = run_bass_kernel_spmd(nc, in_maps, core_ids=list(range(NCORES)))
    pt_sum = 0.0
    ll_sum = 0.0
    for r in res.results:
        o = np.asarray(r["out"], np.float64)
        pt_sum += o[0:CP].sum()
        ll_sum += o[CP:224].sum()
    # padded images contribute |0 - s| = s to the len loss each
    loss = W_LOSS * (np.sqrt(ESCALE) * pt_sum + ll_sum - npad * s) / M_TOTAL
    return np.float32(loss)
